# revision 1
# baseline (speedup 1.0000x reference)
"""CharRNN Trainium2 kernel.

Math: h_{t+1} = tanh(E'[t_s] + h_t @ W_hh.T) with E' = embeddings @ W_ih.T,
then out = h_S @ W_proj.T + b_proj.

Strategy (data-parallel over batch, 8 sequences per core):
- Host precomputes E', one-hot encodings of t, and pre-transposed weight
  layouts; everything is resident in SBUF.
- Recurrent state is kept transposed (hT[p, k*8+b] = h[b, 128k+p]) so it can
  be the stationary matmul operand directly.
- Per step: 18 fp32r matmuls accumulate the tanh-preactivation into PSUM
  (8 hT-chunks x 2 N-halves streaming W_hh.T + 2 one-hot matmuls streaming
  E'), ACT applies tanh, 8 PE transposes + 4 DVE copies rebuild hT.
  Transposes are emitted in pairs separated by matmuls so their
  128-column LDWEIGHTS loads prefetch under adjacent matmul streams,
  each pair in its own PSUM bank.
- Fully unrolled over the 512 steps; the per-step emission order is
  hand-interleaved so every cross-engine chain hides under independent
  matmuls and the PE never stalls (see the order comment in _build).
- Final projection on device, with b_proj folded in via a constant ones-row
  K-chunk.
"""

import numpy as np

import concourse.tile as tile
from concourse import bacc, mybir
from concourse.bass_utils import run_bass_kernel_spmd
from concourse.masks import make_identity

N_CHAR, EMBED, HIDDEN = 128, 256, 1024
BATCH, SEQ = 64, 512
NCORES = 8
BL = BATCH // NCORES  # batch per core
KC = HIDDEN // 128  # K chunks

_cache = {}


def _build():
    f32 = mybir.dt.float32
    f32r = mybir.dt.float32r
    nc = bacc.Bacc(
        "TRN2",
        target_bir_lowering=False,
        debug=False,
        enable_asserts=False,
        num_devices=NCORES,
    )
    wt_d = nc.dram_tensor("wt", [128, KC, HIDDEN], f32r, kind="ExternalInput").ap()
    ep_d = nc.dram_tensor("ep", [128, HIDDEN], f32r, kind="ExternalInput").ap()
    oh_d = nc.dram_tensor("oh", [128, SEQ, BL], f32r, kind="ExternalInput").ap()
    wp_d = nc.dram_tensor("wp", [128, KC + 1, 2 * N_CHAR], f32r, kind="ExternalInput").ap()
    ones_d = nc.dram_tensor("ones_row", [128, BL], f32r, kind="ExternalInput").ap()
    h0t_d = nc.dram_tensor("h0T", [128, KC * BL], f32r, kind="ExternalInput").ap()
    out_d = nc.dram_tensor("out", [BL, N_CHAR], f32, kind="ExternalOutput").ap()

    with tile.TileContext(nc) as tc:
        with (
            tc.tile_pool(name="const", bufs=1) as cpool,
            tc.tile_pool(name="work", bufs=2) as wpool,
            tc.tile_pool(name="psum", bufs=2, space="PSUM") as ppool,
        ):
            # DMAs split per chunk/slice so step 0 can start as soon as its
            # operands land (subtile deps), instead of waiting ~20us for the
            # full 6.5MB preload
            h_a = cpool.tile([128, KC * BL], f32r, name="h_a")
            h_b = cpool.tile([128, KC * BL], f32r, name="h_b")
            nc.sync.dma_start(h_a, h0t_d)
            ep = cpool.tile([128, HIDDEN], f32r, name="ep_sb")
            nc.sync.dma_start(ep, ep_d)
            oh_sb = cpool.tile([128, SEQ, BL], f32r, name="oh_sb")
            nc.sync.dma_start(oh_sb[:, 0:32, :], oh_d[:, 0:32, :])
            # wt delivered half-N first: step 0's ps0 matmuls consume the
            # first 512 columns of every chunk before any second half, so
            # ordering the DMAs [all h0 halves, then h1] feeds step 0 ~5us
            # earlier than whole-chunk-serial delivery
            wt = cpool.tile([128, KC, HIDDEN], f32r, name="wt_sb")
            for half in range(2):
                hsl = slice(512 * half, 512 * (half + 1))
                for k in range(KC):
                    nc.sync.dma_start(wt[:, k, hsl], wt_d[:, k, hsl])
            for sl in range(32, SEQ, 96):
                nc.sync.dma_start(
                    oh_sb[:, sl : sl + 96, :], oh_d[:, sl : sl + 96, :]
                )
            wp = cpool.tile([128, KC + 1, 2 * N_CHAR], f32r, name="wp_sb")
            nc.sync.dma_start(wp, wp_d)
            onesr = cpool.tile([128, BL], f32r, name="ones_sb")
            nc.sync.dma_start(onesr, ones_d)
            ident = cpool.tile([BL, BL], f32, name="ident_sb")
            make_identity(nc, ident)

            tanh = mybir.ActivationFunctionType.Tanh

            def emit_t(hs, tp, half, pair):
                # transpose 2 h chunks into hT layout in a dedicated
                # single-bank psum tile per (half, pair): pairs are emitted
                # with a matmul between them so each transpose's 128-column
                # LDWEIGHTS prefetches into the background weight slot under
                # the preceding matmul's stream instead of serializing, and
                # separate banks keep one pair's DVE copy from serializing
                # against the other pair's PE writes.
                for c in range(2):
                    cc = half * 4 + 2 * pair + c
                    nc.tensor.transpose(
                        tp[:, c * BL : (c + 1) * BL],
                        hs[:, cc * 128 : (cc + 1) * 128],
                        ident,
                    )

            def emit_copy(tp, dst, half, pair):
                lo = (half * 4 + 2 * pair) * BL
                nc.vector.tensor_copy(dst[:, lo : lo + 2 * BL], tp)

            # Fully unrolled over SEQ (static onehot offsets). PE emission
            # order per step is hand-interleaved so every cross-engine chain
            # (tanh -> transposes -> hT copy -> consuming matmuls, each hop
            # costing ~150-200ns of semaphore latency) is covered by enough
            # independent matmuls that the PE never stalls:
            #   [n0 k0-3] [T4-7(j-1)] [n0 oh, n1 k0, n1 k1] [n0 k4-7]
            #   [n1 k2, k3, oh, k4] [T0-3(j)] [n1 k5-7]
            pend = None  # (hs, tp1, dst) of prev step, half-1 untransposed
            n0, n1 = slice(0, 512), slice(512, 1024)
            for j in range(SEQ):
                src = h_a if j % 2 == 0 else h_b
                dst = h_b if j % 2 == 0 else h_a
                # separate PSUM tiles per N-half: ACT reading half 0 must
                # not serialize against PE writing half 1 (same-tile
                # ordering in Tile), so each half gets its own bank
                ps0 = ppool.tile([BL, 512], f32, name="ps0", tag="ps0", bufs=2)
                ps1 = ppool.tile([BL, 512], f32, name="ps1", tag="ps1", bufs=2)
                tp0a = ppool.tile([128, 2 * BL], f32, name="tp0a", tag="tp0a", bufs=1)
                tp0b = ppool.tile([128, 2 * BL], f32, name="tp0b", tag="tp0b", bufs=1)
                tp1a = ppool.tile([128, 2 * BL], f32, name="tp1a", tag="tp1a", bufs=1)
                tp1b = ppool.tile([128, 2 * BL], f32, name="tp1b", tag="tp1b", bufs=1)
                hs = wpool.tile([BL, HIDDEN], f32, name="hs", bufs=4)

                def mmk(ps, nsl, k, start=False, stop=False):
                    nc.tensor.matmul(
                        ps,
                        lhsT=src[:, k * BL : (k + 1) * BL],
                        rhs=wt[:, k, nsl],
                        start=start,
                        stop=stop,
                    )

                # Periodic order (see header): onehots lead (h-independent
                # cover), each act-gate sits 5 MMs before its transpose
                # group, each hT copy gets >=3 MMs of cover before its first
                # consumer.
                nc.tensor.matmul(ps0, lhsT=oh_sb[:, j, :], rhs=ep[:, n0],
                                 start=True, stop=False)
                nc.tensor.matmul(ps1, lhsT=oh_sb[:, j, :], rhs=ep[:, n1],
                                 start=True, stop=False)
                mmk(ps0, n0, 0)
                mmk(ps0, n0, 1)
                mmk(ps0, n0, 2)
                if pend is not None:
                    emit_t(pend[0], pend[1], 1, 0)
                    emit_copy(pend[1], pend[3], 1, pair=0)
                mmk(ps0, n0, 3)
                if pend is not None:
                    emit_t(pend[0], pend[2], 1, 1)
                    emit_copy(pend[2], pend[3], 1, pair=1)
                    pend = None
                mmk(ps1, n1, 0)
                mmk(ps1, n1, 1)
                mmk(ps0, n0, 4)
                mmk(ps0, n0, 5)
                mmk(ps0, n0, 6)
                mmk(ps0, n0, 7, stop=True)
                nc.scalar.activation(hs[:, n0], ps0, tanh)
                mmk(ps1, n1, 2)
                mmk(ps1, n1, 3)
                mmk(ps1, n1, 4)
                mmk(ps1, n1, 5)
                mmk(ps1, n1, 6)
                emit_t(hs, tp0a, 0, 0)
                emit_copy(tp0a, dst, 0, pair=0)
                mmk(ps1, n1, 7, stop=True)
                emit_t(hs, tp0b, 0, 1)
                emit_copy(tp0b, dst, 0, pair=1)
                nc.scalar.activation(hs[:, n1], ps1, tanh)
                pend = (hs, tp1a, tp1b, dst)

            # final projection: out = h_S @ W_proj.T + b_proj (b_proj folded
            # in via the ones-row chunk). SEQ/U iterations of U steps each end
            # with the state back in h_a.
            # projection rhs zero-padded to N=256: fp32r streams at
            # 1 cyc/row only for moving dim >= 256 (4 cyc/row below)
            po = ppool.tile([BL, 2 * N_CHAR], f32, name="po", tag="tp0a", bufs=1)
            for k in range(4):
                nc.tensor.matmul(
                    po,
                    lhsT=h_a[:, k * BL : (k + 1) * BL],
                    rhs=wp[:, k, :],
                    start=(k == 0),
                    stop=False,
                )
            # flush the last step's half-1 transposes between the projection
            # chunks that don't need them and those that do
            emit_t(pend[0], pend[1], 1, 0)
            emit_copy(pend[1], pend[3], 1, pair=0)
            emit_t(pend[0], pend[2], 1, 1)
            emit_copy(pend[2], pend[3], 1, pair=1)
            for k in range(4, KC):
                nc.tensor.matmul(
                    po,
                    lhsT=h_a[:, k * BL : (k + 1) * BL],
                    rhs=wp[:, k, :],
                    start=False,
                    stop=False,
                )
            nc.tensor.matmul(
                po,
                lhsT=onesr,
                rhs=wp[:, KC, :],
                start=False,
                stop=True,
            )
            res = wpool.tile([BL, N_CHAR], f32, name="res")
            nc.vector.tensor_copy(res, po[:, :N_CHAR])
            nc.sync.dma_start(out_d, res)

    nc.compile()
    return nc


def _prep_inputs(t, embeddings, W_ih, W_hh, h0, W_proj, b_proj):
    t = np.asarray(t)
    embeddings = np.asarray(embeddings, dtype=np.float32)
    W_ih = np.asarray(W_ih, dtype=np.float32)
    W_hh = np.asarray(W_hh, dtype=np.float32)
    h0 = np.asarray(h0, dtype=np.float32)
    W_proj = np.asarray(W_proj, dtype=np.float32)
    b_proj = np.asarray(b_proj, dtype=np.float32)

    ep = np.ascontiguousarray(embeddings @ W_ih.T)  # [N_CHAR, HIDDEN]
    # wt[p, k, n] = W_hh.T[128k+p, n]
    wt = np.ascontiguousarray(W_hh.T.reshape(KC, 128, HIDDEN).transpose(1, 0, 2))
    # wp[p, k, j] = W_proj.T[128k+p, j]; extra chunk row 0 carries b_proj
    wp = np.zeros((128, KC + 1, 2 * N_CHAR), dtype=np.float32)
    wp[:, :KC, :N_CHAR] = W_proj.T.reshape(KC, 128, N_CHAR).transpose(1, 0, 2)
    wp[0, KC, :N_CHAR] = b_proj
    ones_row = np.zeros((128, BL), dtype=np.float32)
    ones_row[0, :] = 1.0
    h0f = h0.reshape(HIDDEN)
    h0t = np.ascontiguousarray(
        np.broadcast_to(
            h0f.reshape(KC, 128).T[:, :, None], (128, KC, BL)
        ).reshape(128, KC * BL)
    )

    in_maps = []
    bb, ss = np.meshgrid(np.arange(BL), np.arange(SEQ), indexing="ij")
    for c in range(NCORES):
        tc_ = t[c * BL : (c + 1) * BL, :]  # [BL, SEQ]
        oh = np.zeros((N_CHAR, SEQ, BL), dtype=np.float32)
        oh[tc_[bb, ss], ss, bb] = 1.0
        in_maps.append(
            {
                "wt": wt,
                "ep": ep,
                "oh": oh,
                "wp": wp,
                "ones_row": ones_row,
                "h0T": h0t,
            }
        )
    return in_maps


def _get_nc():
    if "nc" not in _cache:
        _cache["nc"] = _build()
    return _cache["nc"]


def run(trace=False, **inputs):
    nc = _get_nc()
    in_maps = _prep_inputs(**inputs)
    result = run_bass_kernel_spmd(
        nc, in_maps, core_ids=list(range(NCORES)), trace=trace
    )
    out = np.concatenate([r["out"] for r in result.results], axis=0)
    return out, result


def kernel(**inputs) -> np.ndarray:
    out, _ = run(trace=False, **inputs)
    return out



# revision 4
# speedup vs baseline: 3.7995x; 3.7995x over previous
"""CharRNN Trainium2 kernel.

Math: h_{t+1} = tanh(E'[t_s] + h_t @ W_hh.T) with E' = embeddings @ W_ih.T,
then out = h_S @ W_proj.T + b_proj.

Strategy (data-parallel over batch, 8 sequences per core), fp16 weights/state:
- The recurrent state is kept ONLY in transposed layout
  hT[p, k*8+b] = h[b, 128k+p], and each step's matmuls are weight-stationary:
  out-chunk c of the preactivation (transposed, [128, 8]) is accumulated in
  PSUM by one one-hot seed matmul (E' chunk stationary, one-hot column moving)
  plus 8 W_hh.T block matmuls (W block [128,128] stationary, hT chunk [128,8]
  moving). The moving dim is just 8 columns, and the PSUM result is already
  in hT layout, so there are no PE transposes and no DVE copies at all.
- fp16 operands stream at 1 row/cycle at any moving size (fp32r would incur
  a 4x penalty below 256 columns); numerically fp16 weights/state give
  ~8e-4 final rel err (tanh is contracting, W_hh orthogonal).
- Per step: 72 matmuls (free dim 8 each) + 2 tanh ACTs (PSUM -> SBUF fp16,
  one per 4-chunk half) so consuming matmuls of the next step can start as
  soon as their half of the state is ready. Emission order per step:
    [seeds c0-7] [k0-3 x c0-7] [k4-7 x c0-3] ACT0 [k4-7 x c4-7] ACT1
  so the next step's seeds (h-independent) fill the PE while ACT0 lands.
- Fully unrolled over the 512 steps; final projection is done transposed
  (W_proj chunks stationary) with b_proj folded in via a K=1 matmul.
"""

import numpy as np

import concourse.tile as tile
from concourse import bacc, mybir
from concourse.bass_utils import run_bass_kernel_spmd

N_CHAR, EMBED, HIDDEN = 128, 256, 1024
BATCH, SEQ = 64, 512
NCORES = 8
BL = BATCH // NCORES  # batch per core
KC = HIDDEN // 128  # 128-row chunks of the hidden dim

_cache = {}


def _build():
    f16 = mybir.dt.float16
    f32 = mybir.dt.float32
    nc = bacc.Bacc(
        "TRN2",
        target_bir_lowering=False,
        debug=False,
        enable_asserts=False,
        num_devices=NCORES,
    )
    wt_d = nc.dram_tensor("wt", [128, KC, HIDDEN], f16, kind="ExternalInput").ap()
    ep_d = nc.dram_tensor("ep", [128, HIDDEN], f16, kind="ExternalInput").ap()
    oh_d = nc.dram_tensor("oh", [128, SEQ, BL], f16, kind="ExternalInput").ap()
    wpj_d = nc.dram_tensor("wpj", [128, KC, N_CHAR], f16, kind="ExternalInput").ap()
    bp_d = nc.dram_tensor("bp", [1, N_CHAR], f16, kind="ExternalInput").ap()
    ones_d = nc.dram_tensor("ones1", [1, BL], f16, kind="ExternalInput").ap()
    h0t_d = nc.dram_tensor("h0T", [128, KC * BL], f16, kind="ExternalInput").ap()
    out_d = nc.dram_tensor("out", [N_CHAR, BL], f32, kind="ExternalOutput").ap()

    tanh = mybir.ActivationFunctionType.Tanh

    with tile.TileContext(nc) as tc:
        with (
            tc.tile_pool(name="const", bufs=1) as cpool,
            tc.tile_pool(name="work", bufs=2) as wpool,
            tc.tile_pool(name="psum", bufs=2, space="PSUM") as ppool,
        ):
            # DMAs split so step 0 can start as soon as its operands land
            # (subtile deps) instead of waiting for the full preload.
            h0sb = cpool.tile([128, KC * BL], f16, name="h0sb")
            nc.sync.dma_start(h0sb, h0t_d)
            ep = cpool.tile([128, HIDDEN], f16, name="ep_sb")
            nc.sync.dma_start(ep, ep_d)
            oh = cpool.tile([128, SEQ, BL], f16, name="oh_sb")
            nc.sync.dma_start(oh[:, 0:32, :], oh_d[:, 0:32, :])
            wt = cpool.tile([128, KC, HIDDEN], f16, name="wt_sb")
            for k in range(KC):
                nc.sync.dma_start(wt[:, k, :], wt_d[:, k, :])
            for sl in range(32, SEQ, 96):
                nc.sync.dma_start(oh[:, sl : sl + 96, :], oh_d[:, sl : sl + 96, :])
            wpj = cpool.tile([128, KC, N_CHAR], f16, name="wpj_sb")
            nc.sync.dma_start(wpj, wpj_d)
            bp = cpool.tile([1, N_CHAR], f16, name="bp_sb")
            nc.sync.dma_start(bp, bp_d)
            ones1 = cpool.tile([1, BL], f16, name="ones_sb")
            nc.sync.dma_start(ones1, ones_d)

            prev = h0sb
            for j in range(SEQ):
                # one full psum bank per half: start=True zeroes a whole 2KB
                # "zero region", so each half gets exactly one start (its
                # first seed) and one stop (its last k=7 matmul), and the
                # ACT reads a closed bank.
                ps0 = ppool.tile([128, 512], f32, name="ps0", tag="ps0", bufs=2)
                ps1 = ppool.tile([128, 512], f32, name="ps1", tag="ps1", bufs=2)
                hT = wpool.tile([128, KC * BL], f16, name="hT", tag="hT", bufs=3)

                def pslice(c):
                    ps = ps0 if c < 4 else ps1
                    cc = c % 4
                    return ps[:, cc * BL : (cc + 1) * BL]

                def seed(c):
                    nc.tensor.matmul(
                        pslice(c),
                        lhsT=ep[:, c * 128 : (c + 1) * 128],
                        rhs=oh[:, j, :],
                        start=(c % 4 == 0),
                        stop=False,
                    )

                def mm(c, k):
                    nc.tensor.matmul(
                        pslice(c),
                        lhsT=wt[:, k, c * 128 : (c + 1) * 128],
                        rhs=prev[:, k * BL : (k + 1) * BL],
                        start=False,
                        stop=(k == KC - 1 and c % 4 == 3),
                    )

                for c in range(8):
                    seed(c)
                for k in range(4):
                    for c in range(8):
                        mm(c, k)
                for k in range(4, 8):
                    for c in range(4):
                        mm(c, k)
                nc.scalar.activation(hT[:, 0 : 4 * BL], ps0[:, 0 : 4 * BL], tanh)
                for k in range(4, 8):
                    for c in range(4, 8):
                        mm(c, k)
                nc.scalar.activation(hT[:, 4 * BL :], ps1[:, 0 : 4 * BL], tanh)
                prev = hT

            # final projection, transposed: po[j, b] = sum_n W_proj.T[n, j]
            # * h_S[b, n] + b_proj[j] (bias via a K=1 matmul against ones).
            po = ppool.tile([128, 512], f32, name="po", tag="po", bufs=1)
            po = po[:, 0:BL]
            for k in range(KC):
                nc.tensor.matmul(
                    po,
                    lhsT=wpj[:, k, :],
                    rhs=prev[:, k * BL : (k + 1) * BL],
                    start=(k == 0),
                    stop=False,
                )
            nc.tensor.matmul(po, lhsT=bp, rhs=ones1, start=False, stop=True)
            res = wpool.tile([128, BL], f32, name="res")
            nc.vector.tensor_copy(res, po)
            nc.sync.dma_start(out_d, res)

    nc.compile()
    return nc


def _prep_inputs(t, embeddings, W_ih, W_hh, h0, W_proj, b_proj):
    t = np.asarray(t)
    embeddings = np.asarray(embeddings, dtype=np.float32)
    W_ih = np.asarray(W_ih, dtype=np.float32)
    W_hh = np.asarray(W_hh, dtype=np.float32)
    h0 = np.asarray(h0, dtype=np.float32)
    W_proj = np.asarray(W_proj, dtype=np.float32)
    b_proj = np.asarray(b_proj, dtype=np.float32)

    ep = (embeddings @ W_ih.T).astype(np.float16)  # [N_CHAR, HIDDEN]
    # wt[p, k, n] = W_hh.T[128k+p, n]
    wt = np.ascontiguousarray(
        W_hh.T.reshape(KC, 128, HIDDEN).transpose(1, 0, 2)
    ).astype(np.float16)
    # wpj[p, k, j] = W_proj.T[128k+p, j]
    wpj = np.ascontiguousarray(
        W_proj.T.reshape(KC, 128, N_CHAR).transpose(1, 0, 2)
    ).astype(np.float16)
    bp = b_proj.reshape(1, N_CHAR).astype(np.float16)
    ones1 = np.ones((1, BL), dtype=np.float16)
    h0f = h0.reshape(HIDDEN)
    h0t = np.ascontiguousarray(
        np.broadcast_to(
            h0f.reshape(KC, 128).T[:, :, None], (128, KC, BL)
        ).reshape(128, KC * BL)
    ).astype(np.float16)

    in_maps = []
    bb, ss = np.meshgrid(np.arange(BL), np.arange(SEQ), indexing="ij")
    for c in range(NCORES):
        tc_ = t[c * BL : (c + 1) * BL, :]  # [BL, SEQ]
        oh = np.zeros((N_CHAR, SEQ, BL), dtype=np.float16)
        oh[tc_[bb, ss], ss, bb] = 1.0
        in_maps.append(
            {
                "wt": wt,
                "ep": ep,
                "oh": oh,
                "wpj": wpj,
                "bp": bp,
                "ones1": ones1,
                "h0T": h0t,
            }
        )
    return in_maps


def _get_nc():
    if "nc" not in _cache:
        _cache["nc"] = _build()
    return _cache["nc"]


def run(trace=False, **inputs):
    nc = _get_nc()
    in_maps = _prep_inputs(**inputs)
    result = run_bass_kernel_spmd(
        nc, in_maps, core_ids=list(range(NCORES)), trace=trace
    )
    out = np.concatenate([r["out"].T for r in result.results], axis=0)
    return out, result


def kernel(**inputs) -> np.ndarray:
    out, _ = run(trace=False, **inputs)
    return out


# revision 17
# speedup vs baseline: 4.0836x; 1.0748x over previous
"""CharRNN Trainium2 kernel.

Math: h_{t+1} = tanh(E'[t_s] + h_t @ W_hh.T) with E' = embeddings @ W_ih.T,
then out = h_S @ W_proj.T + b_proj.

Strategy (data-parallel over batch, 8 sequences per core), fp16 weights/state:
- The recurrent state is kept ONLY in transposed layout
  hT_s[p, k*bl+b] = h[b, 128k+p] and each step's matmuls are weight-stationary:
  preactivation chunk c (transposed, [128, bl]) is accumulated in PSUM by one
  one-hot seed matmul (E' chunk stationary, one-hot columns moving) plus 8
  W_hh.T block matmuls (W block [128,128] stationary, hT chunk [128,bl]
  moving). The moving dim is tiny and the PSUM result is already in hT
  layout: no PE transposes, no DVE copies.
- fp16 operands stream at 1 row/cycle at any moving size; numerically fp16
  weights/state give ~8e-4 final rel err (tanh contracts, W_hh orthogonal).
- The per-core batch of 8 is processed as SPLITS independent recurrences of
  bl = 8/SPLITS sequences. Each split has its own serial chain
  (matmuls -> tanh ACT -> next-step matmuls, ~700ns of mostly fixed
  cross-engine latency); the splits' chains interleave on the PE/ACT
  engines so the wall-clock per step is one split's chain, not their sum.
- Per split-step: 72 matmuls + ONE tanh ACT (PSUM -> SBUF fp16). Emission:
  [seeds all splits] [K_A] ACT_A [K_B] ACT_B ... so seeds (h-independent)
  execute inside the chain stalls.
- Fully unrolled over the 512 steps; final projection is done transposed
  (W_proj chunks stationary) with b_proj folded in via a K=1 matmul.
"""

import numpy as np

import concourse.tile as tile
from concourse import bacc, mybir
from concourse.bass_utils import run_bass_kernel_spmd

N_CHAR, EMBED, HIDDEN = 128, 256, 1024
BATCH, SEQ = 64, 512
NCORES = 8
BL = BATCH // NCORES  # batch per core
KC = HIDDEN // 128  # 128-row chunks of the hidden dim
SPLITS = 1  # independent sub-recurrences per core (1 measured fastest)

_cache = {}


def _build(splits=SPLITS, seq=SEQ):
    bl = BL // splits
    f16 = mybir.dt.float16
    f32 = mybir.dt.float32
    nc = bacc.Bacc(
        "TRN2",
        target_bir_lowering=False,
        debug=False,
        enable_asserts=False,
        num_devices=NCORES,
    )
    wt_d = nc.dram_tensor("wt", [128, KC, HIDDEN], f16, kind="ExternalInput").ap()
    ep_d = nc.dram_tensor("ep", [128, HIDDEN], f16, kind="ExternalInput").ap()
    oh_d = nc.dram_tensor("oh", [128, SEQ, BL], f16, kind="ExternalInput").ap()
    wpj_d = nc.dram_tensor("wpj", [128, KC, N_CHAR], f16, kind="ExternalInput").ap()
    bp_d = nc.dram_tensor("bp", [1, N_CHAR], f16, kind="ExternalInput").ap()
    ones_d = nc.dram_tensor("ones1", [1, BL], f16, kind="ExternalInput").ap()
    h0t_d = nc.dram_tensor(
        "h0T", [128, splits, KC * bl], f16, kind="ExternalInput"
    ).ap()
    out_d = nc.dram_tensor("out", [N_CHAR, BL], f32, kind="ExternalOutput").ap()

    tanh = mybir.ActivationFunctionType.Tanh

    with tile.TileContext(nc) as tc:
        with (
            tc.tile_pool(name="const", bufs=1) as cpool,
            tc.tile_pool(name="work", bufs=2) as wpool,
            tc.tile_pool(name="psum", bufs=2, space="PSUM") as ppool,
        ):
            # DMAs split so step 0 can start as soon as its operands land
            # (subtile deps) instead of waiting for the full preload.
            # h0T arrives per split: h0sb[:, s, k*bl+b] = h0[128k+p].
            h0sb = cpool.tile([128, splits, KC * bl], f16, name="h0sb")
            nc.sync.dma_start(h0sb, h0t_d)
            ep = cpool.tile([128, HIDDEN], f16, name="ep_sb")
            nc.sync.dma_start(ep, ep_d)
            oh = cpool.tile([128, SEQ, BL], f16, name="oh_sb")
            nc.sync.dma_start(oh[:, 0:32, :], oh_d[:, 0:32, :])
            wt = cpool.tile([128, KC, HIDDEN], f16, name="wt_sb")
            for k in range(KC):
                nc.sync.dma_start(wt[:, k, :], wt_d[:, k, :])
            for sl in range(32, SEQ, 96):
                nc.sync.dma_start(oh[:, sl : sl + 96, :], oh_d[:, sl : sl + 96, :])
            wpj = cpool.tile([128, KC, N_CHAR], f16, name="wpj_sb")
            nc.sync.dma_start(wpj, wpj_d)
            bp = cpool.tile([1, N_CHAR], f16, name="bp_sb")
            nc.sync.dma_start(bp, bp_d)
            ones1 = cpool.tile([1, BL], f16, name="ones_sb")
            nc.sync.dma_start(ones1, ones_d)

            prev = [h0sb[:, s, :] for s in range(splits)]
            for j in range(seq):
                hTs = []
                pss = []
                for s in range(splits):
                    ps = ppool.tile(
                        [128, 512], f32, name=f"ps{s}", tag=f"ps{s}",
                        bufs=(4 if splits <= 2 else 2),
                    )
                    # a unique buffer per step: no WAW/WAR on the hT tile, so
                    # the tanh ACT carries a single (PE data) wait that embeds
                    # into the instruction instead of a SEQ-holding
                    # EventSemaphore on the Activation sequencer.
                    hT = wpool.tile(
                        [128, KC * bl], f16, name=f"hT{s}", tag=f"hT{s}",
                        bufs=seq + 1,
                    )
                    pss.append(ps)
                    hTs.append(hT)

                # seeds for all splits first: they have no h dependency and
                # execute inside the cross-engine chain stalls.
                for s in range(splits):
                    for c in range(KC):
                        nc.tensor.matmul(
                            pss[s][:, c * bl : (c + 1) * bl],
                            lhsT=ep[:, c * 128 : (c + 1) * 128],
                            rhs=oh[:, j, s * bl : (s + 1) * bl],
                            start=(c == 0),
                            stop=False,
                        )
                for s in range(splits):
                    ps = pss[s]
                    pv = prev[s]
                    for k in range(KC):
                        for c in range(KC):
                            nc.tensor.matmul(
                                ps[:, c * bl : (c + 1) * bl],
                                lhsT=wt[:, k, c * 128 : (c + 1) * 128],
                                rhs=pv[:, k * bl : (k + 1) * bl],
                                start=False,
                                stop=(k == KC - 1 and c == KC - 1),
                            )
                    nc.scalar.activation(hTs[s], ps[:, 0 : KC * bl], tanh)
                    if j == 0 and s + 1 < splits:
                        # stagger the splits' chains: the next split's initial
                        # state arrives via an ACT-engine copy emitted after
                        # this split's first tanh, so its whole chain runs
                        # ~half a period out of phase and fills this split's
                        # latency stalls instead of synchronizing with it.
                        d = wpool.tile(
                            [128, KC * bl], f16,
                            name=f"h0d{s + 1}", tag=f"hT{s + 1}", bufs=seq + 1,
                        )
                        nc.scalar.activation(
                            d, h0sb[:, s + 1, :],
                            mybir.ActivationFunctionType.Copy,
                        )
                        prev[s + 1] = d
                prev = hTs

            # final projection, transposed: po[j, b] = sum_n W_proj.T[n, j]
            # * h_S[b, n] + b_proj[j] (bias via a K=1 matmul against ones).
            po = ppool.tile(
                [128, 512], f32, name="po", tag="ps0",
                bufs=(4 if splits <= 2 else 2),
            )
            first = True
            for k in range(KC):
                for s in range(splits):
                    nc.tensor.matmul(
                        po[:, s * bl : (s + 1) * bl],
                        lhsT=wpj[:, k, :],
                        rhs=prev[s][:, k * bl : (k + 1) * bl],
                        start=first,
                        stop=False,
                    )
                    first = False
            nc.tensor.matmul(
                po[:, 0:BL], lhsT=bp, rhs=ones1, start=False, stop=True
            )
            res = wpool.tile([128, BL], f32, name="res")
            nc.vector.tensor_copy(res, po[:, 0:BL])
            nc.sync.dma_start(out_d, res)

    nc.compile()
    return nc


def _prep_inputs(t, embeddings, W_ih, W_hh, h0, W_proj, b_proj, splits=SPLITS):
    bl = BL // splits
    t = np.asarray(t)
    embeddings = np.asarray(embeddings, dtype=np.float32)
    W_ih = np.asarray(W_ih, dtype=np.float32)
    W_hh = np.asarray(W_hh, dtype=np.float32)
    h0 = np.asarray(h0, dtype=np.float32)
    W_proj = np.asarray(W_proj, dtype=np.float32)
    b_proj = np.asarray(b_proj, dtype=np.float32)

    ep = (embeddings @ W_ih.T).astype(np.float16)  # [N_CHAR, HIDDEN]
    # wt[p, k, n] = W_hh.T[128k+p, n]
    wt = np.ascontiguousarray(
        W_hh.T.reshape(KC, 128, HIDDEN).transpose(1, 0, 2)
    ).astype(np.float16)
    # wpj[p, k, j] = W_proj.T[128k+p, j]
    wpj = np.ascontiguousarray(
        W_proj.T.reshape(KC, 128, N_CHAR).transpose(1, 0, 2)
    ).astype(np.float16)
    bp = b_proj.reshape(1, N_CHAR).astype(np.float16)
    ones1 = np.ones((1, BL), dtype=np.float16)
    # h0T[p, s, k*bl+b] = h0[128k+p]  (identical for every split/batch col)
    h0f = h0.reshape(HIDDEN)
    h0t = np.ascontiguousarray(
        np.broadcast_to(
            h0f.reshape(KC, 128).T[None, :, :, None], (splits, 128, KC, bl)
        ).transpose(1, 0, 2, 3).reshape(128, splits, KC * bl)
    ).astype(np.float16)

    # per-core output batch order: split-major (split s holds batch rows
    # [s*bl, (s+1)*bl) of the core's 8)
    in_maps = []
    bb, ss = np.meshgrid(np.arange(BL), np.arange(SEQ), indexing="ij")
    for c in range(NCORES):
        tc_ = t[c * BL : (c + 1) * BL, :]  # [BL, SEQ]
        oh = np.zeros((N_CHAR, SEQ, BL), dtype=np.float16)
        oh[tc_[bb, ss], ss, bb] = 1.0
        in_maps.append(
            {
                "wt": wt,
                "ep": ep,
                "oh": oh,
                "wpj": wpj,
                "bp": bp,
                "ones1": ones1,
                "h0T": h0t,
            }
        )
    return in_maps


def _get_nc():
    if "nc" not in _cache:
        _cache["nc"] = _build()
    return _cache["nc"]


def run(trace=False, **inputs):
    nc = _get_nc()
    in_maps = _prep_inputs(**inputs)
    result = run_bass_kernel_spmd(
        nc, in_maps, core_ids=list(range(NCORES)), trace=trace
    )
    out = np.concatenate([r["out"].T for r in result.results], axis=0)
    return out, result


def kernel(**inputs) -> np.ndarray:
    out, _ = run(trace=False, **inputs)
    return out


# revision 24
# speedup vs baseline: 4.6303x; 1.1339x over previous
"""CharRNN Trainium2 kernel.

Math: h_{t+1} = tanh(E'[t_s] + h_t @ W_hh.T) with E' = embeddings @ W_ih.T,
then out = h_S @ W_proj.T + b_proj.

Strategy (data-parallel over batch, 8 sequences per core):
- The recurrent state is kept ONLY in transposed layout
  hT[p, k, b] = h[b, 128k+p] and each step's matmuls are weight-stationary:
  preactivation chunk c (transposed, [128, 8]) is accumulated in PSUM by one
  one-hot seed matmul (E' chunk stationary, one-hot columns moving) plus
  W_hh.T block matmuls (W block stationary, hT chunk moving). The moving dim
  is just 8 columns and the PSUM result is already in hT layout: no PE
  transposes, no DVE copies. One tanh ACT per step (PSUM -> SBUF).
- The recurrence is contractive (orthogonal W_hh, tanh' < 1): errors
  injected at step j decay exponentially by step 511, so the first T0=448
  steps run with fp8e4m3 weights/state using DoubleRow matmuls (2 k-tiles
  per instruction, 0.5 cycles/row -> 32 PE instructions/step instead of 64)
  and only the last 64 steps use fp16 weights/state, which pulls the
  trajectory back to full accuracy. Measured end-to-end rel err 1.7e-3
  (vs 8.5e-4 all-fp16) against the 2e-2 gate.
- Every step's hT gets a unique SBUF buffer, so the tanh ACT carries a
  single (PE data) wait that embeds into the instruction; with recycled
  buffers the extra WAW wait forced a SEQ-holding EventSemaphore that added
  ~80ns/step to the serial chain.
- Steady-state step time is chain-latency-bound:
  ACT exec (238) + ACT->PE visibility (240) + K matmuls + PE->ACT drain
  (209); the fp8 phase shrinks the K term from ~247 to ~140ns.
- Fully unrolled over the 512 steps; final projection is done transposed
  (W_proj chunks stationary) with b_proj folded in via a K=1 matmul.
"""

import ml_dtypes
import numpy as np

import concourse.tile as tile
from concourse import bacc, mybir
from concourse.bass_utils import run_bass_kernel_spmd

N_CHAR, EMBED, HIDDEN = 128, 256, 1024
BATCH, SEQ = 64, 512
NCORES = 8
BL = BATCH // NCORES  # batch per core
KC = HIDDEN // 128  # 128-row chunks of the hidden dim
SPLITS = 1  # independent sub-recurrences per core (1 measured fastest)
T0 = 448  # steps run in fp8e4m3 (rest in fp16)

_cache = {}


def _build(splits=SPLITS, seq=SEQ, t0=T0):
    assert 1 <= t0 < seq
    bl = BL // splits
    f8 = mybir.dt.float8e4
    f16 = mybir.dt.float16
    f32 = mybir.dt.float32
    dr = mybir.MatmulPerfMode.DoubleRow
    nc = bacc.Bacc(
        "TRN2",
        target_bir_lowering=False,
        debug=False,
        enable_asserts=False,
        num_devices=NCORES,
    )
    wt_d = nc.dram_tensor("wt", [128, KC, HIDDEN], f16, kind="ExternalInput").ap()
    wt8_d = nc.dram_tensor("wt8", [128, KC, HIDDEN], f8, kind="ExternalInput").ap()
    ep_d = nc.dram_tensor("ep", [128, HIDDEN], f16, kind="ExternalInput").ap()
    oh_d = nc.dram_tensor("oh", [128, SEQ, BL], f16, kind="ExternalInput").ap()
    wpj_d = nc.dram_tensor("wpj", [128, KC, N_CHAR], f16, kind="ExternalInput").ap()
    bp_d = nc.dram_tensor("bp", [1, N_CHAR], f16, kind="ExternalInput").ap()
    ones_d = nc.dram_tensor("ones1", [1, BL], f16, kind="ExternalInput").ap()
    h0t_d = nc.dram_tensor(
        "h0T", [128, splits * KC, bl], f8, kind="ExternalInput"
    ).ap()
    out_d = nc.dram_tensor("out", [N_CHAR, BL], f32, kind="ExternalOutput").ap()

    tanh = mybir.ActivationFunctionType.Tanh

    with tile.TileContext(nc) as tc:
        with (
            tc.tile_pool(name="const", bufs=1) as cpool,
            tc.tile_pool(name="work", bufs=2) as wpool,
            tc.tile_pool(name="psum", bufs=2, space="PSUM") as ppool,
        ):
            # Preload: step 0 needs h0 + ep + oh col 0 + all of wt8; the
            # fp16 wt is not consumed until step T0 (~370us in), so it
            # streams in last. DMAs are split so step 0 starts on subtile
            # deps instead of the full preload.
            h0sb = cpool.tile([128, splits * KC, bl], f8, name="h0sb")
            nc.sync.dma_start(h0sb, h0t_d)
            ep = cpool.tile([128, HIDDEN], f16, name="ep_sb")
            nc.sync.dma_start(ep, ep_d)
            oh = cpool.tile([128, SEQ, BL], f16, name="oh_sb")
            nc.sync.dma_start(oh[:, 0:32, :], oh_d[:, 0:32, :])
            wt8 = cpool.tile([128, KC, HIDDEN], f8, name="wt8_sb")
            for k in range(KC):
                nc.sync.dma_start(wt8[:, k, :], wt8_d[:, k, :])
            for sl in range(32, SEQ, 96):
                nc.sync.dma_start(oh[:, sl : sl + 96, :], oh_d[:, sl : sl + 96, :])
            wpj = cpool.tile([128, KC, N_CHAR], f16, name="wpj_sb")
            nc.sync.dma_start(wpj, wpj_d)
            bp = cpool.tile([1, N_CHAR], f16, name="bp_sb")
            nc.sync.dma_start(bp, bp_d)
            ones1 = cpool.tile([1, BL], f16, name="ones_sb")
            nc.sync.dma_start(ones1, ones_d)
            wt = cpool.tile([128, KC, HIDDEN], f16, name="wt_sb")
            for k in range(KC):
                nc.sync.dma_start(wt[:, k, :], wt_d[:, k, :])

            prev = [h0sb[:, s * KC : (s + 1) * KC, :] for s in range(splits)]
            for j in range(seq):
                phase8 = j < t0
                out8 = j < t0 - 1  # step j+1 consumes fp8 state
                hTs = []
                pss = []
                for s in range(splits):
                    ps = ppool.tile(
                        [128, 512], f32, name=f"ps{s}", tag=f"ps{s}", bufs=4
                    )
                    # a unique buffer per step: no WAW/WAR on the hT tile,
                    # so the tanh ACT carries a single embeddable PE wait.
                    if out8:
                        hT = wpool.tile(
                            [128, KC, bl], f8, name=f"h8_{s}", tag=f"h8_{s}",
                            bufs=t0,
                        )
                    else:
                        hT = wpool.tile(
                            [128, KC, bl], f16, name=f"hT{s}", tag=f"hT{s}",
                            bufs=seq - t0 + 2,
                        )
                    pss.append(ps)
                    hTs.append(hT)

                # seeds for all splits first: no h dependency, they execute
                # inside the cross-engine chain stalls.
                for s in range(splits):
                    for c in range(KC):
                        nc.tensor.matmul(
                            pss[s][:, c * bl : (c + 1) * bl],
                            lhsT=ep[:, c * 128 : (c + 1) * 128],
                            rhs=oh[:, j, s * bl : (s + 1) * bl],
                            start=(c == 0),
                            stop=False,
                        )
                for s in range(splits):
                    ps = pss[s]
                    pv = prev[s]
                    if phase8:
                        for kk in range(KC // 2):
                            for c in range(KC):
                                nc.tensor.matmul(
                                    ps[:, c * bl : (c + 1) * bl],
                                    lhsT=wt8[:, 2 * kk : 2 * kk + 2,
                                             c * 128 : (c + 1) * 128],
                                    rhs=pv[:, 2 * kk : 2 * kk + 2, :],
                                    start=False,
                                    stop=(kk == KC // 2 - 1 and c == KC - 1),
                                    perf_mode=dr,
                                )
                    else:
                        for k in range(KC):
                            for c in range(KC):
                                nc.tensor.matmul(
                                    ps[:, c * bl : (c + 1) * bl],
                                    lhsT=wt[:, k, c * 128 : (c + 1) * 128],
                                    rhs=pv[:, k, :],
                                    start=False,
                                    stop=(k == KC - 1 and c == KC - 1),
                                )
                    nc.scalar.activation(hTs[s], ps[:, 0 : KC * bl], tanh)
                prev = hTs

            # final projection, transposed: po[j, b] = sum_n W_proj.T[n, j]
            # * h_S[b, n] + b_proj[j] (bias via a K=1 matmul against ones).
            po = ppool.tile([128, 512], f32, name="po", tag="ps0", bufs=4)
            first = True
            for k in range(KC):
                for s in range(splits):
                    nc.tensor.matmul(
                        po[:, s * bl : (s + 1) * bl],
                        lhsT=wpj[:, k, :],
                        rhs=prev[s][:, k, :],
                        start=first,
                        stop=False,
                    )
                    first = False
            nc.tensor.matmul(
                po[:, 0:BL], lhsT=bp, rhs=ones1, start=False, stop=True
            )
            res = wpool.tile([128, BL], f32, name="res")
            nc.vector.tensor_copy(res, po[:, 0:BL])
            nc.sync.dma_start(out_d, res)

    nc.compile()
    return nc


def _prep_inputs(t, embeddings, W_ih, W_hh, h0, W_proj, b_proj, splits=SPLITS):
    bl = BL // splits
    t = np.asarray(t)
    embeddings = np.asarray(embeddings, dtype=np.float32)
    W_ih = np.asarray(W_ih, dtype=np.float32)
    W_hh = np.asarray(W_hh, dtype=np.float32)
    h0 = np.asarray(h0, dtype=np.float32)
    W_proj = np.asarray(W_proj, dtype=np.float32)
    b_proj = np.asarray(b_proj, dtype=np.float32)

    ep = (embeddings @ W_ih.T).astype(np.float16)  # [N_CHAR, HIDDEN]
    # wt[p, k, n] = W_hh.T[128k+p, n]
    wtf = np.ascontiguousarray(W_hh.T.reshape(KC, 128, HIDDEN).transpose(1, 0, 2))
    wt = wtf.astype(np.float16)
    wt8 = wtf.astype(ml_dtypes.float8_e4m3fn)
    # wpj[p, k, j] = W_proj.T[128k+p, j]
    wpj = np.ascontiguousarray(
        W_proj.T.reshape(KC, 128, N_CHAR).transpose(1, 0, 2)
    ).astype(np.float16)
    bp = b_proj.reshape(1, N_CHAR).astype(np.float16)
    ones1 = np.ones((1, BL), dtype=np.float16)
    # h0T[p, s*KC + k, b] = h0[128k+p]
    h0f = h0.reshape(HIDDEN)
    h0t = np.ascontiguousarray(
        np.broadcast_to(
            h0f.reshape(KC, 128).T[:, None, :, None], (128, splits, KC, bl)
        ).reshape(128, splits * KC, bl)
    ).astype(ml_dtypes.float8_e4m3fn)

    in_maps = []
    bb, ss = np.meshgrid(np.arange(BL), np.arange(SEQ), indexing="ij")
    for c in range(NCORES):
        tc_ = t[c * BL : (c + 1) * BL, :]  # [BL, SEQ]
        oh = np.zeros((N_CHAR, SEQ, BL), dtype=np.float16)
        oh[tc_[bb, ss], ss, bb] = 1.0
        in_maps.append(
            {
                "wt": wt,
                "wt8": wt8,
                "ep": ep,
                "oh": oh,
                "wpj": wpj,
                "bp": bp,
                "ones1": ones1,
                "h0T": h0t,
            }
        )
    return in_maps


def _get_nc():
    if "nc" not in _cache:
        _cache["nc"] = _build()
    return _cache["nc"]


def run(trace=False, **inputs):
    nc = _get_nc()
    in_maps = _prep_inputs(**inputs)
    result = run_bass_kernel_spmd(
        nc, in_maps, core_ids=list(range(NCORES)), trace=trace
    )
    out = np.concatenate([r["out"].T for r in result.results], axis=0)
    return out, result


def kernel(**inputs) -> np.ndarray:
    out, _ = run(trace=False, **inputs)
    return out
